# revision 43
# baseline (speedup 1.0000x reference)
"""Multi-head attention (pre-LN + residual) on 8 trn2 NeuronCores.

Sharding: core r = (batch b = r//4, head group i = r%4, 4 heads each).
v2 design: fp8(e4m3) datapath with DoubleRow (double-pumped) matmuls for
the QKV/V/AV/w_o GEMMs, softmax exp split across ScalarE (exact exp),
VectorE and GpSimd (Schraudolph bit-trick exp writing fp8 bits via a
uint8-bitcast tensor_scalar), bf16 x input to halve the startup DMA,
and one small fp8 AllToAll per 512-query chunk (output rows strided
across cores so every chunk addresses all 8 destination slots), each
overlapped with the next chunk's attention; the output projection for
chunk qc is interleaved behind chunk qc+1.
"""

import sys

sys.path.insert(0, "/opt/trn_rl_repo")

import numpy as np
import ml_dtypes

BF16 = ml_dtypes.bfloat16
E4M3 = ml_dtypes.float8_e4m3  # TRN fp8e4 (IEEE-ish, max 240)

# Problem constants (hardcoded per contract)
B = 2
S = 2048
D = 1024
H = 16
DK = 64
NCORES = 8
HLOC = 4  # heads per core
DLOC = HLOC * DK  # 256
SLICE = S // NCORES  # 256 output rows per batch per core
EPS = 1e-5
SCALE = 1.0 / np.sqrt(DK)
WS = 32.0  # fp8 weight scale
ESC = float(SCALE / (WS * WS))  # exp input scale (psum -> true score)
SCHR_A = float((8.0 / np.log(2.0)) * ESC)  # Schraudolph fp8: mult const
SCHR_B = 56.5  # Schraudolph fp8: add const (incl +0.5 trunc bias)

ST = S // 128  # 16 seq tiles
FT = D // 128  # 8 feature tiles
QC = S // 512  # 4 q-chunks

# exp engine schedule per 16 k-tiles: A=ScalarE exact exp, D=DVE schraudolph
EXP_PAT = "ADADADADADADADAA"

_CACHE = {}


def _row_idx(r):
    """Global q rows owned by out-core r, qc-major: Q = 512*qc + 64*r + k."""
    return (np.arange(QC)[:, None] * 512 + 64 * r + np.arange(64)[None, :]).reshape(-1)


def _build():
    import concourse.bass as bass
    import concourse.mybir as mybir
    import concourse.tile as tile
    from concourse import bacc
    from concourse.masks import make_identity

    dt = mybir.dt
    AF = mybir.ActivationFunctionType
    OP = mybir.AluOpType
    DR = mybir.MatmulPerfMode.DoubleRow

    nc = bacc.Bacc(
        "TRN2",
        target_bir_lowering=False,
        debug=False,
        enable_asserts=False,
        num_devices=NCORES,
    )

    # ---- I/O ----
    x_b = nc.dram_tensor("x_b", [S, D], dt.bfloat16, kind="ExternalInput").ap()
    wqT = nc.dram_tensor("wqT", [D, DLOC], dt.float8e4, kind="ExternalInput").ap()
    wkT = nc.dram_tensor("wkT", [D, DLOC], dt.float8e4, kind="ExternalInput").ap()
    wvT = nc.dram_tensor("wvT", [D, DLOC], dt.float8e4, kind="ExternalInput").ap()
    woT = nc.dram_tensor("woT", [D, D], dt.float8e4, kind="ExternalInput").ap()
    x_res = nc.dram_tensor(
        "x_res", [B, SLICE, D], dt.float32, kind="ExternalInput"
    ).ap()
    b_o = nc.dram_tensor("b_o", [D], dt.float32, kind="ExternalInput").ap()
    out_sl = nc.dram_tensor(
        "out_sl", [B, SLICE, D], dt.float32, kind="ExternalOutput"
    ).ap()

    with tile.TileContext(nc) as tc:
        with (
            tc.tile_pool(name="singles", bufs=1) as singles,
            tc.tile_pool(name="persist", bufs=1) as persist,
            tc.tile_pool(name="dram", bufs=1, space="DRAM") as dram,
        ):
            ident = singles.tile([128, 128], dt.float8e4)
            make_identity(nc, ident)
            eps_t = singles.tile([128, 1], dt.float32)
            nc.vector.memset(eps_t, EPS)
            ones_r = singles.tile([1, DK], dt.float32)
            nc.vector.memset(ones_r, 1.0)

            # ---- persistent intermediates ----
            xnt_t = persist.tile([128, FT, S], dt.float8e4, tag="xnt", name="xnt")
            qT = [
                persist.tile([128, S], dt.float8e4, tag=f"qT{m}", name=f"qT{m}")
                for m in range(2)
            ]
            kT = [
                persist.tile([128, S], dt.float8e4, tag=f"kT{m}", name=f"kT{m}")
                for m in range(2)
            ]
            vp_t = persist.tile(
                [128, HLOC, ST, 96], dt.float8e4, tag="vp", name="vp"
            )
            nc.gpsimd.memset(vp_t[:, :, :, DK : DK + 1], 1.0)
            nc.gpsimd.memset(vp_t[:, :, :, DK + 1 : 96], 0.0)

            # collective bounce buffers: group 0 = qc0+qc1, 1 = qc2, 2 = qc3
            A2A_W = {0: 2, 1: 1, 2: 1}
            a2a_in = {
                g: dram.tile(
                    [NCORES, 4 * DK, A2A_W[g], 64], dt.float8e4,
                    name=f"a2a_in{g}", tag=f"a2a_in{g}",
                )
                for g in range(3)
            }
            a2a_out = {
                g: dram.tile(
                    [NCORES, 4 * DK, A2A_W[g], 64], dt.float8e4,
                    name=f"a2a_out{g}", tag=f"a2a_out{g}",
                )
                for g in range(3)
            }

            wq_sb = singles.tile([128, FT, DLOC], dt.float8e4)
            wk_sb = singles.tile([128, FT, DLOC], dt.float8e4)
            wv_sb = singles.tile([128, FT, DLOC], dt.float8e4)
            b_bc = singles.tile([128, D], dt.float32)
            wo_sb = singles.tile([128, FT, D], dt.float8e4)
            # residual+bias rows: partitions = (b, 64-row-block), free = (qc, D)
            xrb = singles.tile([128, QC, D], dt.float32)

            x_rows = x_b.rearrange("(t p) d -> t p d", p=128)

            with (
                tc.tile_pool(name="ln", bufs=10) as ln_pool,
                tc.tile_pool(name="lnst", bufs=10) as lnst,
                tc.tile_pool(name="epool", bufs=12) as epool,
                tc.tile_pool(name="aopool", bufs=6) as aopool,
                tc.tile_pool(name="ivpool", bufs=6) as ivpool,
                tc.tile_pool(name="attg", bufs=4) as attg_pool,
                tc.tile_pool(name="outp", bufs=6) as outp,
            ):

                def emit_ln(st, ps_tr):
                    x_t = ln_pool.tile([128, D], dt.bfloat16, tag="x", name="x_t")
                    nc.sync.dma_start(out=x_t, in_=x_rows[st])
                    stats = lnst.tile(
                        [128, 2, 6], dt.float32, tag="stats", name="stats"
                    )
                    for g in range(2):
                        nc.vector.bn_stats(
                            out=stats[:, g, :], in_=x_t[:, g * 512 : (g + 1) * 512]
                        )
                    mv = lnst.tile([128, 2], dt.float32, tag="mv", name="mv")
                    nc.vector.bn_aggr(out=mv, in_=stats)
                    sd = lnst.tile([128, 1], dt.float32, tag="sd", name="sd")
                    nc.scalar.activation(
                        out=sd, in_=mv[:, 1:2], func=AF.Sqrt, bias=eps_t, scale=1.0
                    )
                    rinv = lnst.tile([128, 1], dt.float32, tag="rinv", name="rinv")
                    nc.vector.reciprocal_approx_fast(out=rinv, in_=sd)
                    negmur = lnst.tile(
                        [128, 1], dt.float32, tag="negmur", name="negmur"
                    )
                    nc.vector.tensor_scalar(
                        out=negmur,
                        in0=mv[:, 0:1],
                        scalar1=rinv,
                        scalar2=-1.0,
                        op0=OP.mult,
                        op1=OP.mult,
                    )
                    xn = ln_pool.tile([128, D], dt.float8e4, tag="xn", name="xn")
                    if st % 4 == 3:
                        nc.vector.tensor_scalar(
                            out=xn,
                            in0=x_t,
                            scalar1=rinv,
                            scalar2=negmur,
                            op0=OP.mult,
                            op1=OP.add,
                        )
                    else:
                        nc.scalar.activation(
                            out=xn, in_=x_t, func=AF.Identity, bias=negmur, scale=rinv
                        )
                    # fp8 transpose writes PSUM with element step 2
                    tr = ps_tr.tile([128, FT, 128, 2], dt.float8e4, tag="tr", name="tr")
                    for fp in range(FT):
                        nc.tensor.transpose(
                            tr[:, fp, :, 0],
                            xn[:, fp * 128 : (fp + 1) * 128],
                            ident,
                        )
                    ev = [nc.scalar.copy, nc.vector.tensor_copy][st % 2]
                    ev(
                        out=xnt_t[:, :, st * 128 : (st + 1) * 128],
                        in_=tr[:, :, :, 0],
                    )

                def emit_kq(w_sb, dst, hp, ch, ps_kqv):
                    ps = ps_kqv.tile([128, 512], dt.float32, tag="kq", name="kq_ps")
                    for fp in range(FT // 2):
                        nc.tensor.matmul(
                            ps,
                            lhsT=w_sb[:, 2 * fp : 2 * fp + 2, hp * 128 : (hp + 1) * 128],
                            rhs=xnt_t[:, 2 * fp : 2 * fp + 2, ch * 512 : (ch + 1) * 512],
                            start=(fp == 0),
                            stop=(fp == FT // 2 - 1),
                            perf_mode=DR,
                        )
                    ev = [nc.scalar.copy, nc.vector.tensor_copy][(2 * ch + hp) % 2]
                    ev(out=dst[hp][:, ch * 512 : (ch + 1) * 512], in_=ps)

                def emit_v(st, ps_kqv):
                    ps = ps_kqv.tile([128, DLOC], dt.float32, tag="v", name="v_ps")
                    for fp in range(FT // 2):
                        nc.tensor.matmul(
                            ps,
                            lhsT=xnt_t[:, 2 * fp : 2 * fp + 2, st * 128 : (st + 1) * 128],
                            rhs=wv_sb[:, 2 * fp : 2 * fp + 2, :],
                            start=(fp == 0),
                            stop=(fp == FT // 2 - 1),
                            perf_mode=DR,
                        )
                    nc.scalar.copy(
                        out=vp_t[:, :, st, 0:DK],
                        in_=ps.rearrange("p (h d) -> p h d", h=HLOC),
                    )

                def emit_scores(hp, qc, kt, e2, slot, ps_s):
                    s_ps = ps_s.tile([128, 1024], dt.float32, tag="s", name="s_ps")
                    for j in range(2):
                        nc.tensor.matmul(
                            s_ps[:, j * 512 : (j + 1) * 512],
                            lhsT=kT[hp][
                                j * 64 : (j + 1) * 64, kt * 128 : (kt + 1) * 128
                            ],
                            rhs=qT[hp][
                                j * 64 : (j + 1) * 64, qc * 512 : (qc + 1) * 512
                            ],
                            start=True,
                            stop=True,
                        )
                    eng = EXP_PAT[kt]
                    dst = e2[:, slot, :]
                    if eng == "A":
                        nc.scalar.activation(out=dst, in_=s_ps, func=AF.Exp, scale=ESC)
                    else:
                        e = nc.vector if eng == "D" else nc.gpsimd
                        e.tensor_scalar(
                            out=dst.bitcast(dt.uint8),
                            in0=s_ps,
                            scalar1=SCHR_A,
                            scalar2=SCHR_B,
                            op0=OP.mult,
                            op1=OP.add,
                        )

                def emit_av_j(hp, j, ktp, av, e2):
                    nc.tensor.matmul(
                        av,
                        lhsT=vp_t[:, 2 * hp + j, ktp : ktp + 2, :],
                        rhs=e2[:, :, j * 512 : (j + 1) * 512],
                        start=(ktp == 0),
                        stop=(ktp == ST - 2),
                        perf_mode=DR,
                    )

                def emit_normalize_j(hp, qc, j, av):
                    # evict av fast (frees the PSUM bank); broadcast 1/den
                    # across partitions with a rank-1 PE matmul (no GpSimd,
                    # whose queue is occupied by the collectives)
                    den = ivpool.tile([1, 512], dt.float32, tag="den", name="den")
                    nc.vector.tensor_copy(out=den, in_=av[DK : DK + 1, :])
                    avs = aopool.tile([DK, 512], dt.float32, tag="avs", name="avs")
                    nc.scalar.copy(out=avs, in_=av[0:DK, :])
                    invd = ivpool.tile([1, 512], dt.float32, tag="invd", name="invd")
                    nc.vector.reciprocal_approx_fast(out=invd, in_=den)
                    ibc = ps_wo.tile([128, 512], dt.float32, tag="wo", name="ibc")
                    nc.tensor.matmul(
                        ibc[0:DK, :],
                        lhsT=ones_r,
                        rhs=invd,
                        start=True,
                        stop=True,
                    )
                    ao = aopool.tile([DK, 512], dt.float8e4, tag="ao", name="ao")
                    nc.vector.tensor_mul(out=ao, in0=avs, in1=ibc[0:DK, :])
                    # rows hp*128 + j*64 + dk of each dest slot
                    g, sub = (0, qc) if qc < 2 else (qc - 1, 0)
                    dma_eng = nc.scalar if qc == 3 else nc.sync
                    dma_eng.dma_start(
                        out=a2a_in[g][
                            :, hp * 128 + j * DK : hp * 128 + (j + 1) * DK, sub, :
                        ].rearrange("s d q -> d s q"),
                        in_=ao.rearrange("d (s q) -> d s q", s=NCORES),
                    )

                def attn_hp(hp, qc, ps_av, ps_s):
                    av = ps_av.tile([96, 512], dt.float32, tag="av", name=f"av{hp}0")
                    done = []
                    pend = []
                    for ktp in range(0, ST, 2):
                        e2 = epool.tile(
                            [128, 2, 1024], dt.float8e4, tag="e2", name="e2"
                        )
                        emit_scores(hp, qc, ktp, e2, 0, ps_s)
                        emit_scores(hp, qc, ktp + 1, e2, 1, ps_s)
                        pend.append((ktp, e2))
                        if len(pend) > 1:
                            pk, pe = pend.pop(0)
                            emit_av_j(hp, 0, pk, av, pe)
                            done.append((pk, pe))
                    for pk, pe in pend:
                        emit_av_j(hp, 0, pk, av, pe)
                        done.append((pk, pe))
                    emit_normalize_j(hp, qc, 0, av)
                    av1 = ps_av.tile([96, 512], dt.float32, tag="av", name=f"av{hp}1")
                    for pk, pe in done:
                        emit_av_j(hp, 1, pk, av1, pe)
                    emit_normalize_j(hp, qc, 1, av1)

                def emit_a2a(g):
                    nc.gpsimd.collective_compute(
                        "AllToAll",
                        mybir.AluOpType.bypass,
                        replica_groups=[list(range(NCORES))],
                        ins=[a2a_in[g].opt()],
                        outs=[a2a_out[g].opt()],
                    )

                def emit_wo(qc, ps_wo):
                    # lhsT free = (ksub, b, 64q) -> out partitions (b, 64q)
                    ag = attg_pool.tile(
                        [128, FT, B, 64], dt.float8e4, tag="ag", name="ag"
                    )
                    g, sub = (0, qc) if qc < 2 else (qc - 1, 0)
                    for b in range(B):
                        nc.sync.dma_start(
                            out=ag[:, :, b, :],
                            in_=a2a_out[g][
                                4 * b : 4 * (b + 1), :, sub, :
                            ].rearrange("s (t p) q -> p (s t) q", p=128),
                        )
                    for oc in range(2):
                        ps = ps_wo.tile(
                            [128, 512], dt.float32, tag="wo", name=f"wo{oc}"
                        )
                        for sp in range(FT // 2):
                            nc.tensor.matmul(
                                ps,
                                lhsT=ag[:, 2 * sp : 2 * sp + 2, :, :],
                                rhs=wo_sb[
                                    :, 2 * sp : 2 * sp + 2,
                                    oc * 512 : (oc + 1) * 512,
                                ],
                                start=(sp == 0),
                                stop=(sp == FT // 2 - 1),
                                perf_mode=DR,
                            )
                        o_t = outp.tile([128, 512], dt.float32, tag="o", name="o_t")
                        nc.vector.scalar_tensor_tensor(
                            out=o_t,
                            in0=ps,
                            scalar=float(1.0 / (WS * WS)),
                            in1=xrb[:, qc, oc * 512 : (oc + 1) * 512],
                            op0=OP.mult,
                            op1=OP.add,
                        )
                        for b in range(B):
                            nc.sync.dma_start(
                                out=out_sl[
                                    b,
                                    qc * 64 : (qc + 1) * 64,
                                    oc * 512 : (oc + 1) * 512,
                                ],
                                in_=o_t[64 * b : 64 * (b + 1), :],
                            )

                # ===== Phase A: LN + transposes + K/Q/V projections ========
                ps_kqv_cm = tc.tile_pool(name="ps_kqv", bufs=2, space="PSUM")
                ps_kqv = ps_kqv_cm.__enter__()
                ps_tr_cm = tc.tile_pool(name="ps_tr", bufs=2, space="PSUM")
                ps_tr = ps_tr_cm.__enter__()
                for c in range(4):
                    for st in range(4 * c, 4 * c + 4):
                        emit_ln(st, ps_tr)
                    if c == 0:
                        nc.sync.dma_start(
                            out=wk_sb, in_=wkT.rearrange("(t p) m -> p t m", p=128)
                        )
                        nc.sync.dma_start(
                            out=wq_sb, in_=wqT.rearrange("(t p) m -> p t m", p=128)
                        )
                        nc.sync.dma_start(
                            out=wv_sb, in_=wvT.rearrange("(t p) m -> p t m", p=128)
                        )
                    for hp in range(2):
                        emit_kq(wk_sb, kT, hp, c, ps_kqv)
                    for hp in range(2):
                        emit_kq(wq_sb, qT, hp, c, ps_kqv)
                    for st in range(4 * c, 4 * c + 4):
                        emit_v(st, ps_kqv)
                    if c == 1:
                        nc.sync.dma_start(
                            out=b_bc,
                            in_=bass.AP(
                                tensor=b_o.tensor,
                                offset=b_o.offset,
                                ap=[[0, 128]] + list(b_o.ap),
                            ),
                        )
                        nc.sync.dma_start(
                            out=wo_sb, in_=woT.rearrange("(t p) m -> p t m", p=128)
                        )
                        for b in range(B):
                            nc.sync.dma_start(
                                out=xrb[64 * b : 64 * (b + 1), :, :],
                                in_=x_res[b].rearrange("(t p) d -> p t d", p=64),
                            )
                for t in range(QC):
                    nc.gpsimd.tensor_add(
                        out=xrb[:, t, :], in0=xrb[:, t, :], in1=b_bc
                    )
                ps_tr_cm.__exit__(None, None, None)
                ps_kqv_cm.__exit__(None, None, None)

                # ===== Phase B/C: attention, per-qc AllToAll, wo ==========
                ps_s_cm = tc.tile_pool(name="ps_s", bufs=3, space="PSUM")
                ps_s = ps_s_cm.__enter__()
                ps_av_cm = tc.tile_pool(name="ps_av", bufs=1, space="PSUM")
                ps_av = ps_av_cm.__enter__()
                ps_wo_cm = tc.tile_pool(name="ps_wo", bufs=1, space="PSUM")
                ps_wo = ps_wo_cm.__enter__()

                for qc in range(QC):
                    for hp in range(2):
                        attn_hp(hp, qc, ps_av, ps_s)
                        if qc == 3 and hp == 0:
                            emit_wo(0, ps_wo)
                        if qc == 3 and hp == 1:
                            emit_wo(1, ps_wo)
                    if qc == 1:
                        emit_a2a(0)
                    elif qc == 2:
                        emit_a2a(1)
                    elif qc == 3:
                        emit_wo(2, ps_wo)
                        emit_a2a(2)
                emit_wo(3, ps_wo)

                ps_wo_cm.__exit__(None, None, None)
                ps_av_cm.__exit__(None, None, None)
                ps_s_cm.__exit__(None, None, None)

    nc.compile()
    return nc


def _get_nc():
    if "nc" not in _CACHE:
        _CACHE["nc"] = _build()
    return _CACHE["nc"]


def _make_in_maps(inputs):
    x = np.asarray(inputs["x"], np.float32)
    w_q = np.asarray(inputs["w_q"], np.float32)
    w_k = np.asarray(inputs["w_k"], np.float32)
    w_v = np.asarray(inputs["w_v"], np.float32)
    w_o = np.asarray(inputs["w_o"], np.float32)
    b_o = np.asarray(inputs["b_o"], np.float32)
    gamma = np.asarray(inputs["ln_gamma"], np.float32)
    beta = np.asarray(inputs["ln_beta"], np.float32)

    assert np.allclose(beta, 0.0), "nonzero ln_beta not supported"
    woT = np.ascontiguousarray((w_o * WS).T).astype(E4M3)
    # LN gamma folds exactly into the input side of the QKV projections
    w_qg = w_q * gamma[None, :] * WS
    w_kg = w_k * gamma[None, :] * WS
    w_vg = w_v * gamma[None, :] * WS
    in_maps = []
    for r in range(NCORES):
        b, i = r // 4, r % 4
        sl = slice(DLOC * i, DLOC * (i + 1))
        rows = _row_idx(r)
        in_maps.append(
            {
                "x_b": np.ascontiguousarray(x[b]).astype(BF16),
                "wqT": np.ascontiguousarray(w_qg[sl].T).astype(E4M3),
                "wkT": np.ascontiguousarray(w_kg[sl].T).astype(E4M3),
                "wvT": np.ascontiguousarray(w_vg[sl].T).astype(E4M3),
                "woT": woT,
                "x_res": np.ascontiguousarray(x[:, rows, :]),
                "b_o": b_o,
            }
        )
    return in_maps


def _install_ntff_hook():
    """The agent image's antenv lacks axon_hooks; recreate it so
    trace=True can capture NTFF profiles through libaxon_pjrt.so."""
    import types

    from concourse import bass_utils

    if "antenv.axon_hooks" not in sys.modules:
        import antenv
        from trn_agent_boot.trn_boot import _ntff_profile_via_ctypes

        mod = types.ModuleType("antenv.axon_hooks")
        state = {}
        mod.set_axon_ntff_profile_hook = lambda h: state.update(h=h)
        mod.get_axon_ntff_profile_hook = lambda: state.get("h")
        sys.modules["antenv.axon_hooks"] = mod
        antenv.axon_hooks = mod
        mod.set_axon_ntff_profile_hook(
            _ntff_profile_via_ctypes("/opt/axon/libaxon_pjrt.so")
        )
    bass_utils.upload_artifacts = lambda tmpdir: tmpdir


def run(inputs, trace=False, tmpdir=None, trace_cores=None):
    from concourse import bass_utils

    if trace:
        _install_ntff_hook()
    nc = _get_nc()
    in_maps = _make_in_maps(inputs)
    res = bass_utils.run_bass_kernel_spmd(
        nc,
        in_maps,
        core_ids=list(range(NCORES)),
        trace=trace,
        tmpdir=tmpdir,
        trace_cores=trace_cores,
    )
    out = np.empty((B, S, D), np.float32)
    for r in range(NCORES):
        out[:, _row_idx(r), :] = res.results[r]["out_sl"]
    return out, res


def kernel(**inputs):
    out, _ = run(inputs)
    return out


# revision 44
# speedup vs baseline: 1.0212x; 1.0212x over previous
"""Multi-head attention (pre-LN + residual) on 8 trn2 NeuronCores.

Sharding: core r = (batch b = r//4, head group i = r%4, 4 heads each).
v2 design: fp8(e4m3) datapath with DoubleRow (double-pumped) matmuls for
the QKV/V/AV/w_o GEMMs, softmax exp split across ScalarE (exact exp),
VectorE and GpSimd (Schraudolph bit-trick exp writing fp8 bits via a
uint8-bitcast tensor_scalar), bf16 x input to halve the startup DMA,
and one small fp8 AllToAll per 512-query chunk (output rows strided
across cores so every chunk addresses all 8 destination slots), each
overlapped with the next chunk's attention; the output projection for
chunk qc is interleaved behind chunk qc+1.
"""

import sys

sys.path.insert(0, "/opt/trn_rl_repo")

import numpy as np
import ml_dtypes

BF16 = ml_dtypes.bfloat16
E4M3 = ml_dtypes.float8_e4m3  # TRN fp8e4 (IEEE-ish, max 240)

# Problem constants (hardcoded per contract)
B = 2
S = 2048
D = 1024
H = 16
DK = 64
NCORES = 8
HLOC = 4  # heads per core
DLOC = HLOC * DK  # 256
SLICE = S // NCORES  # 256 output rows per batch per core
EPS = 1e-5
SCALE = 1.0 / np.sqrt(DK)
WS = 32.0  # fp8 weight scale
ESC = float(SCALE / (WS * WS))  # exp input scale (psum -> true score)
SCHR_A = float((8.0 / np.log(2.0)) * ESC)  # Schraudolph fp8: mult const
SCHR_B = 56.5  # Schraudolph fp8: add const (incl +0.5 trunc bias)

ST = S // 128  # 16 seq tiles
FT = D // 128  # 8 feature tiles
QC = S // 512  # 4 q-chunks

# exp engine schedule per 16 k-tiles: A=ScalarE exact exp, D=DVE schraudolph
EXP_PAT = "ADADADADADADADAA"

_CACHE = {}


def _row_idx(r):
    """Global q rows owned by out-core r, qc-major: Q = 512*qc + 64*r + k."""
    return (np.arange(QC)[:, None] * 512 + 64 * r + np.arange(64)[None, :]).reshape(-1)


def _build():
    import concourse.bass as bass
    import concourse.mybir as mybir
    import concourse.tile as tile
    from concourse import bacc
    from concourse.masks import make_identity

    dt = mybir.dt
    AF = mybir.ActivationFunctionType
    OP = mybir.AluOpType
    DR = mybir.MatmulPerfMode.DoubleRow

    nc = bacc.Bacc(
        "TRN2",
        target_bir_lowering=False,
        debug=False,
        enable_asserts=False,
        num_devices=NCORES,
    )

    # ---- I/O ----
    x_b = nc.dram_tensor("x_b", [S, D], dt.bfloat16, kind="ExternalInput").ap()
    wqT = nc.dram_tensor("wqT", [D, DLOC], dt.float8e4, kind="ExternalInput").ap()
    wkT = nc.dram_tensor("wkT", [D, DLOC], dt.float8e4, kind="ExternalInput").ap()
    wvT = nc.dram_tensor("wvT", [D, DLOC], dt.float8e4, kind="ExternalInput").ap()
    woT = nc.dram_tensor("woT", [D, D], dt.float8e4, kind="ExternalInput").ap()
    x_res = nc.dram_tensor(
        "x_res", [B, SLICE, D], dt.float32, kind="ExternalInput"
    ).ap()
    b_o = nc.dram_tensor("b_o", [D], dt.float32, kind="ExternalInput").ap()
    out_sl = nc.dram_tensor(
        "out_sl", [B, SLICE, D], dt.float32, kind="ExternalOutput"
    ).ap()

    with tile.TileContext(nc) as tc:
        with (
            tc.tile_pool(name="singles", bufs=1) as singles,
            tc.tile_pool(name="persist", bufs=1) as persist,
            tc.tile_pool(name="dram", bufs=1, space="DRAM") as dram,
        ):
            ident = singles.tile([128, 128], dt.float8e4)
            make_identity(nc, ident)
            eps_t = singles.tile([128, 1], dt.float32)
            nc.vector.memset(eps_t, EPS)
            ones_r = singles.tile([1, DK], dt.float32)
            nc.vector.memset(ones_r, 1.0)

            # ---- persistent intermediates ----
            xnt_t = persist.tile([128, FT, S], dt.float8e4, tag="xnt", name="xnt")
            qT = [
                persist.tile([128, S], dt.float8e4, tag=f"qT{m}", name=f"qT{m}")
                for m in range(2)
            ]
            kT = [
                persist.tile([128, S], dt.float8e4, tag=f"kT{m}", name=f"kT{m}")
                for m in range(2)
            ]
            vp_t = persist.tile(
                [128, HLOC, ST, 96], dt.float8e4, tag="vp", name="vp"
            )
            nc.gpsimd.memset(vp_t[:, :, :, DK : DK + 1], 1.0)
            nc.gpsimd.memset(vp_t[:, :, :, DK + 1 : 96], 0.0)

            # collective bounce buffers: group 0 = qc0+qc1, 1 = qc2, 2 = qc3
            A2A_W = {0: 2, 1: 1, 2: 1}
            a2a_in = {
                g: dram.tile(
                    [NCORES, 4 * DK, A2A_W[g], 64], dt.float8e4,
                    name=f"a2a_in{g}", tag=f"a2a_in{g}",
                )
                for g in range(3)
            }
            a2a_out = {
                g: dram.tile(
                    [NCORES, 4 * DK, A2A_W[g], 64], dt.float8e4,
                    name=f"a2a_out{g}", tag=f"a2a_out{g}",
                )
                for g in range(3)
            }

            wq_sb = singles.tile([128, FT, DLOC], dt.float8e4)
            wk_sb = singles.tile([128, FT, DLOC], dt.float8e4)
            wv_sb = singles.tile([128, FT, DLOC], dt.float8e4)
            b_bc = singles.tile([128, D], dt.float32)
            wo_sb = singles.tile([128, FT, D], dt.float8e4)
            # residual+bias rows: partitions = (b, 64-row-block), free = (qc, D)
            xrb = singles.tile([128, QC, D], dt.float32)

            x_rows = x_b.rearrange("(t p) d -> t p d", p=128)

            with (
                tc.tile_pool(name="ln", bufs=6) as ln_pool,
                tc.tile_pool(name="lnst", bufs=6) as lnst,
                tc.tile_pool(name="epool", bufs=12) as epool,
                tc.tile_pool(name="aopool", bufs=4) as aopool,
                tc.tile_pool(name="ivpool", bufs=4) as ivpool,
                tc.tile_pool(name="attg", bufs=4) as attg_pool,
                tc.tile_pool(name="outp", bufs=4) as outp,
            ):

                def emit_ln(st, ps_tr):
                    x_t = ln_pool.tile([128, D], dt.bfloat16, tag="x", name="x_t")
                    nc.sync.dma_start(out=x_t, in_=x_rows[st])
                    stats = lnst.tile(
                        [128, 2, 6], dt.float32, tag="stats", name="stats"
                    )
                    for g in range(2):
                        nc.vector.bn_stats(
                            out=stats[:, g, :], in_=x_t[:, g * 512 : (g + 1) * 512]
                        )
                    mv = lnst.tile([128, 2], dt.float32, tag="mv", name="mv")
                    nc.vector.bn_aggr(out=mv, in_=stats)
                    sd = lnst.tile([128, 1], dt.float32, tag="sd", name="sd")
                    nc.scalar.activation(
                        out=sd, in_=mv[:, 1:2], func=AF.Sqrt, bias=eps_t, scale=1.0
                    )
                    rinv = lnst.tile([128, 1], dt.float32, tag="rinv", name="rinv")
                    nc.vector.reciprocal_approx_fast(out=rinv, in_=sd)
                    negmur = lnst.tile(
                        [128, 1], dt.float32, tag="negmur", name="negmur"
                    )
                    nc.vector.tensor_scalar(
                        out=negmur,
                        in0=mv[:, 0:1],
                        scalar1=rinv,
                        scalar2=-1.0,
                        op0=OP.mult,
                        op1=OP.mult,
                    )
                    xn = ln_pool.tile([128, D], dt.float8e4, tag="xn", name="xn")
                    if st % 4 == 3:
                        nc.vector.tensor_scalar(
                            out=xn,
                            in0=x_t,
                            scalar1=rinv,
                            scalar2=negmur,
                            op0=OP.mult,
                            op1=OP.add,
                        )
                    else:
                        nc.scalar.activation(
                            out=xn, in_=x_t, func=AF.Identity, bias=negmur, scale=rinv
                        )
                    # fp8 transpose writes PSUM with element step 2
                    tr = ps_tr.tile([128, FT, 128, 2], dt.float8e4, tag="tr", name="tr")
                    for fp in range(FT):
                        nc.tensor.transpose(
                            tr[:, fp, :, 0],
                            xn[:, fp * 128 : (fp + 1) * 128],
                            ident,
                        )
                    ev = [nc.scalar.copy, nc.vector.tensor_copy][st % 2]
                    ev(
                        out=xnt_t[:, :, st * 128 : (st + 1) * 128],
                        in_=tr[:, :, :, 0],
                    )

                def emit_kq(w_sb, dst, hp, ch, ps_kqv):
                    ps = ps_kqv.tile([128, 512], dt.float32, tag="kq", name="kq_ps")
                    for fp in range(FT // 2):
                        nc.tensor.matmul(
                            ps,
                            lhsT=w_sb[:, 2 * fp : 2 * fp + 2, hp * 128 : (hp + 1) * 128],
                            rhs=xnt_t[:, 2 * fp : 2 * fp + 2, ch * 512 : (ch + 1) * 512],
                            start=(fp == 0),
                            stop=(fp == FT // 2 - 1),
                            perf_mode=DR,
                        )
                    ev = [nc.scalar.copy, nc.vector.tensor_copy][(2 * ch + hp) % 2]
                    ev(out=dst[hp][:, ch * 512 : (ch + 1) * 512], in_=ps)

                def emit_v(st, ps_kqv):
                    ps = ps_kqv.tile([128, DLOC], dt.float32, tag="v", name="v_ps")
                    for fp in range(FT // 2):
                        nc.tensor.matmul(
                            ps,
                            lhsT=xnt_t[:, 2 * fp : 2 * fp + 2, st * 128 : (st + 1) * 128],
                            rhs=wv_sb[:, 2 * fp : 2 * fp + 2, :],
                            start=(fp == 0),
                            stop=(fp == FT // 2 - 1),
                            perf_mode=DR,
                        )
                    nc.scalar.copy(
                        out=vp_t[:, :, st, 0:DK],
                        in_=ps.rearrange("p (h d) -> p h d", h=HLOC),
                    )

                def emit_scores(hp, qc, kt, e2, slot, ps_s):
                    s_ps = ps_s.tile([128, 1024], dt.float32, tag="s", name="s_ps")
                    for j in range(2):
                        nc.tensor.matmul(
                            s_ps[:, j * 512 : (j + 1) * 512],
                            lhsT=kT[hp][
                                j * 64 : (j + 1) * 64, kt * 128 : (kt + 1) * 128
                            ],
                            rhs=qT[hp][
                                j * 64 : (j + 1) * 64, qc * 512 : (qc + 1) * 512
                            ],
                            start=True,
                            stop=True,
                        )
                    eng = EXP_PAT[kt]
                    dst = e2[:, slot, :]
                    if eng == "A":
                        nc.scalar.activation(out=dst, in_=s_ps, func=AF.Exp, scale=ESC)
                    else:
                        e = nc.vector if eng == "D" else nc.gpsimd
                        e.tensor_scalar(
                            out=dst.bitcast(dt.uint8),
                            in0=s_ps,
                            scalar1=SCHR_A,
                            scalar2=SCHR_B,
                            op0=OP.mult,
                            op1=OP.add,
                        )

                def emit_av_j(hp, j, ktp, av, e2):
                    nc.tensor.matmul(
                        av,
                        lhsT=vp_t[:, 2 * hp + j, ktp : ktp + 2, :],
                        rhs=e2[:, :, j * 512 : (j + 1) * 512],
                        start=(ktp == 0),
                        stop=(ktp == ST - 2),
                        perf_mode=DR,
                    )

                def emit_normalize_j(hp, qc, j, av):
                    # evict av fast (frees the PSUM bank); broadcast 1/den
                    # across partitions with a rank-1 PE matmul (no GpSimd,
                    # whose queue is occupied by the collectives)
                    den = ivpool.tile([1, 512], dt.float32, tag="den", name="den")
                    nc.vector.tensor_copy(out=den, in_=av[DK : DK + 1, :])
                    avs = aopool.tile([DK, 512], dt.float32, tag="avs", name="avs")
                    nc.scalar.copy(out=avs, in_=av[0:DK, :])
                    invd = ivpool.tile([1, 512], dt.float32, tag="invd", name="invd")
                    nc.vector.reciprocal_approx_fast(out=invd, in_=den)
                    ibc = ps_wo.tile([128, 512], dt.float32, tag="wo", name="ibc")
                    nc.tensor.matmul(
                        ibc[0:DK, :],
                        lhsT=ones_r,
                        rhs=invd,
                        start=True,
                        stop=True,
                    )
                    ao = aopool.tile([DK, 512], dt.float8e4, tag="ao", name="ao")
                    nc.vector.tensor_mul(out=ao, in0=avs, in1=ibc[0:DK, :])
                    # rows hp*128 + j*64 + dk of each dest slot
                    g, sub = (0, qc) if qc < 2 else (qc - 1, 0)
                    dma_eng = nc.scalar if qc == 3 else nc.sync
                    dma_eng.dma_start(
                        out=a2a_in[g][
                            :, hp * 128 + j * DK : hp * 128 + (j + 1) * DK, sub, :
                        ].rearrange("s d q -> d s q"),
                        in_=ao.rearrange("d (s q) -> d s q", s=NCORES),
                    )

                def attn_hp(hp, qc, ps_av, ps_s):
                    av = ps_av.tile([96, 512], dt.float32, tag="av", name=f"av{hp}0")
                    done = []
                    pend = []
                    for ktp in range(0, ST, 2):
                        e2 = epool.tile(
                            [128, 2, 1024], dt.float8e4, tag="e2", name="e2"
                        )
                        emit_scores(hp, qc, ktp, e2, 0, ps_s)
                        emit_scores(hp, qc, ktp + 1, e2, 1, ps_s)
                        pend.append((ktp, e2))
                        if len(pend) > 1:
                            pk, pe = pend.pop(0)
                            emit_av_j(hp, 0, pk, av, pe)
                            done.append((pk, pe))
                    for pk, pe in pend:
                        emit_av_j(hp, 0, pk, av, pe)
                        done.append((pk, pe))
                    emit_normalize_j(hp, qc, 0, av)
                    av1 = ps_av.tile([96, 512], dt.float32, tag="av", name=f"av{hp}1")
                    for pk, pe in done:
                        emit_av_j(hp, 1, pk, av1, pe)
                    emit_normalize_j(hp, qc, 1, av1)

                def emit_a2a(g):
                    nc.gpsimd.collective_compute(
                        "AllToAll",
                        mybir.AluOpType.bypass,
                        replica_groups=[list(range(NCORES))],
                        ins=[a2a_in[g].opt()],
                        outs=[a2a_out[g].opt()],
                    )

                def emit_wo(qc, ps_wo):
                    # lhsT free = (ksub, b, 64q) -> out partitions (b, 64q)
                    ag = attg_pool.tile(
                        [128, FT, B, 64], dt.float8e4, tag="ag", name="ag"
                    )
                    g, sub = (0, qc) if qc < 2 else (qc - 1, 0)
                    for b in range(B):
                        nc.sync.dma_start(
                            out=ag[:, :, b, :],
                            in_=a2a_out[g][
                                4 * b : 4 * (b + 1), :, sub, :
                            ].rearrange("s (t p) q -> p (s t) q", p=128),
                        )
                    for oc in range(2):
                        ps = ps_wo.tile(
                            [128, 512], dt.float32, tag="wo", name=f"wo{oc}"
                        )
                        for sp in range(FT // 2):
                            nc.tensor.matmul(
                                ps,
                                lhsT=ag[:, 2 * sp : 2 * sp + 2, :, :],
                                rhs=wo_sb[
                                    :, 2 * sp : 2 * sp + 2,
                                    oc * 512 : (oc + 1) * 512,
                                ],
                                start=(sp == 0),
                                stop=(sp == FT // 2 - 1),
                                perf_mode=DR,
                            )
                        o_t = outp.tile([128, 512], dt.float32, tag="o", name="o_t")
                        nc.vector.scalar_tensor_tensor(
                            out=o_t,
                            in0=ps,
                            scalar=float(1.0 / (WS * WS)),
                            in1=xrb[:, qc, oc * 512 : (oc + 1) * 512],
                            op0=OP.mult,
                            op1=OP.add,
                        )
                        for b in range(B):
                            nc.sync.dma_start(
                                out=out_sl[
                                    b,
                                    qc * 64 : (qc + 1) * 64,
                                    oc * 512 : (oc + 1) * 512,
                                ],
                                in_=o_t[64 * b : 64 * (b + 1), :],
                            )

                # ===== Phase A: LN + transposes + K/Q/V projections ========
                ps_kqv_cm = tc.tile_pool(name="ps_kqv", bufs=2, space="PSUM")
                ps_kqv = ps_kqv_cm.__enter__()
                ps_tr_cm = tc.tile_pool(name="ps_tr", bufs=2, space="PSUM")
                ps_tr = ps_tr_cm.__enter__()
                for c in range(4):
                    for st in range(4 * c, 4 * c + 4):
                        emit_ln(st, ps_tr)
                    if c == 0:
                        nc.sync.dma_start(
                            out=wk_sb, in_=wkT.rearrange("(t p) m -> p t m", p=128)
                        )
                        nc.sync.dma_start(
                            out=wq_sb, in_=wqT.rearrange("(t p) m -> p t m", p=128)
                        )
                        nc.sync.dma_start(
                            out=wv_sb, in_=wvT.rearrange("(t p) m -> p t m", p=128)
                        )
                    for hp in range(2):
                        emit_kq(wk_sb, kT, hp, c, ps_kqv)
                    for hp in range(2):
                        emit_kq(wq_sb, qT, hp, c, ps_kqv)
                    for st in range(4 * c, 4 * c + 4):
                        emit_v(st, ps_kqv)
                    if c == 1:
                        nc.sync.dma_start(
                            out=b_bc,
                            in_=bass.AP(
                                tensor=b_o.tensor,
                                offset=b_o.offset,
                                ap=[[0, 128]] + list(b_o.ap),
                            ),
                        )
                        nc.sync.dma_start(
                            out=wo_sb, in_=woT.rearrange("(t p) m -> p t m", p=128)
                        )
                        for b in range(B):
                            nc.sync.dma_start(
                                out=xrb[64 * b : 64 * (b + 1), :, :],
                                in_=x_res[b].rearrange("(t p) d -> p t d", p=64),
                            )
                for t in range(QC):
                    nc.gpsimd.tensor_add(
                        out=xrb[:, t, :], in0=xrb[:, t, :], in1=b_bc
                    )
                ps_tr_cm.__exit__(None, None, None)
                ps_kqv_cm.__exit__(None, None, None)

                # ===== Phase B/C: attention, per-qc AllToAll, wo ==========
                ps_s_cm = tc.tile_pool(name="ps_s", bufs=3, space="PSUM")
                ps_s = ps_s_cm.__enter__()
                ps_av_cm = tc.tile_pool(name="ps_av", bufs=1, space="PSUM")
                ps_av = ps_av_cm.__enter__()
                ps_wo_cm = tc.tile_pool(name="ps_wo", bufs=1, space="PSUM")
                ps_wo = ps_wo_cm.__enter__()

                for qc in range(QC):
                    for hp in range(2):
                        attn_hp(hp, qc, ps_av, ps_s)
                        if qc == 3 and hp == 0:
                            emit_wo(0, ps_wo)
                        if qc == 3 and hp == 1:
                            emit_wo(1, ps_wo)
                    if qc == 1:
                        emit_a2a(0)
                    elif qc == 2:
                        emit_a2a(1)
                    elif qc == 3:
                        emit_wo(2, ps_wo)
                        emit_a2a(2)
                emit_wo(3, ps_wo)

                ps_wo_cm.__exit__(None, None, None)
                ps_av_cm.__exit__(None, None, None)
                ps_s_cm.__exit__(None, None, None)

    nc.compile()
    return nc


def _get_nc():
    if "nc" not in _CACHE:
        _CACHE["nc"] = _build()
    return _CACHE["nc"]


def _make_in_maps(inputs):
    x = np.asarray(inputs["x"], np.float32)
    w_q = np.asarray(inputs["w_q"], np.float32)
    w_k = np.asarray(inputs["w_k"], np.float32)
    w_v = np.asarray(inputs["w_v"], np.float32)
    w_o = np.asarray(inputs["w_o"], np.float32)
    b_o = np.asarray(inputs["b_o"], np.float32)
    gamma = np.asarray(inputs["ln_gamma"], np.float32)
    beta = np.asarray(inputs["ln_beta"], np.float32)

    assert np.allclose(beta, 0.0), "nonzero ln_beta not supported"
    woT = np.ascontiguousarray((w_o * WS).T).astype(E4M3)
    # LN gamma folds exactly into the input side of the QKV projections
    w_qg = w_q * gamma[None, :] * WS
    w_kg = w_k * gamma[None, :] * WS
    w_vg = w_v * gamma[None, :] * WS
    in_maps = []
    for r in range(NCORES):
        b, i = r // 4, r % 4
        sl = slice(DLOC * i, DLOC * (i + 1))
        rows = _row_idx(r)
        in_maps.append(
            {
                "x_b": np.ascontiguousarray(x[b]).astype(BF16),
                "wqT": np.ascontiguousarray(w_qg[sl].T).astype(E4M3),
                "wkT": np.ascontiguousarray(w_kg[sl].T).astype(E4M3),
                "wvT": np.ascontiguousarray(w_vg[sl].T).astype(E4M3),
                "woT": woT,
                "x_res": np.ascontiguousarray(x[:, rows, :]),
                "b_o": b_o,
            }
        )
    return in_maps


def _install_ntff_hook():
    """The agent image's antenv lacks axon_hooks; recreate it so
    trace=True can capture NTFF profiles through libaxon_pjrt.so."""
    import types

    from concourse import bass_utils

    if "antenv.axon_hooks" not in sys.modules:
        import antenv
        from trn_agent_boot.trn_boot import _ntff_profile_via_ctypes

        mod = types.ModuleType("antenv.axon_hooks")
        state = {}
        mod.set_axon_ntff_profile_hook = lambda h: state.update(h=h)
        mod.get_axon_ntff_profile_hook = lambda: state.get("h")
        sys.modules["antenv.axon_hooks"] = mod
        antenv.axon_hooks = mod
        mod.set_axon_ntff_profile_hook(
            _ntff_profile_via_ctypes("/opt/axon/libaxon_pjrt.so")
        )
    bass_utils.upload_artifacts = lambda tmpdir: tmpdir


def run(inputs, trace=False, tmpdir=None, trace_cores=None):
    from concourse import bass_utils

    if trace:
        _install_ntff_hook()
    nc = _get_nc()
    in_maps = _make_in_maps(inputs)
    res = bass_utils.run_bass_kernel_spmd(
        nc,
        in_maps,
        core_ids=list(range(NCORES)),
        trace=trace,
        tmpdir=tmpdir,
        trace_cores=trace_cores,
    )
    out = np.empty((B, S, D), np.float32)
    for r in range(NCORES):
        out[:, _row_idx(r), :] = res.results[r]["out_sl"]
    return out, res


def kernel(**inputs):
    out, _ = run(inputs)
    return out
